# revision 10
# baseline (speedup 1.0000x reference)
"""Context-parallel causal attention block on 8 Trainium2 NeuronCores.

Strategy: tensor-parallel split-heads. Each core c computes Q/K/V projections
for its 2 heads (of 16) over all tokens with host-sliced weights, runs causal
attention locally (feature-major layouts, no transposes), then one on-device
AllToAll re-shards from head-parallel to token-parallel, and each core runs the
output projection for its 512-token row slice. Host concatenates row slices.
"""
import sys

sys.path.insert(0, "/opt/trn_rl_repo")

import numpy as np

import concourse.bass as bass
import concourse.tile as tile
from concourse import bacc, mybir
from concourse.bass_utils import run_bass_kernel_spmd

FP = mybir.dt.float32
N_CORES = 8
B, S, D, H, DH = 2, 2048, 2048, 16, 128
T = B * S            # 4096 flattened tokens, b-major
KK = D // 128        # 16 contraction k-tiles
NSTRIP = T // 512    # 8 token strips of 512
ROWS = T // N_CORES  # 512 output rows per core
HPC = H // N_CORES   # 2 heads per core
NEG = -1.0e30


def build_nc(debug_taps: bool = False) -> bacc.Bacc:
    nc = bacc.Bacc("TRN2", target_bir_lowering=False, debug=False, num_devices=N_CORES)

    xt = nc.dram_tensor("xt", [128, KK, T], FP, kind="ExternalInput")
    wq = nc.dram_tensor("wq", [128, KK, 256], FP, kind="ExternalInput")
    wk = nc.dram_tensor("wk", [128, KK, 256], FP, kind="ExternalInput")
    wv = nc.dram_tensor("wv", [128, KK, 256], FP, kind="ExternalInput")
    wo = nc.dram_tensor("wo", [128, KK, D], FP, kind="ExternalInput")
    out_t = nc.dram_tensor("out_t", [D, ROWS], FP, kind="ExternalOutput")
    dbg = {}
    if debug_taps:
        dbg["q"] = nc.dram_tensor("dbg_q", [2, 128, T], FP, kind="ExternalOutput")
        dbg["k"] = nc.dram_tensor("dbg_k", [128, 2, T], FP, kind="ExternalOutput")
        dbg["v"] = nc.dram_tensor("dbg_v", [128, 32, 256], FP, kind="ExternalOutput")
        dbg["ain"] = nc.dram_tensor("dbg_ain", [N_CORES, 256, 512], FP, kind="ExternalOutput")
        dbg["aout"] = nc.dram_tensor("dbg_aout", [N_CORES, 256, 512], FP, kind="ExternalOutput")
        dbg["masks"] = nc.dram_tensor("dbg_masks", [128, 4, 512], FP, kind="ExternalOutput")

    with tile.TileContext(nc) as tc:
        with (
            tc.tile_pool(name="dram", bufs=1, space="DRAM") as dram,
            tc.tile_pool(name="consts", bufs=1) as consts,
            tc.tile_pool(name="persist", bufs=1) as persist,
        ):
            qt_d = dram.tile([2, 128, T], FP)           # qT spill  [hl, dh, t]
            a2a_in = dram.tile([N_CORES, 256, 512], FP)  # [dest, feat, tok]
            a2a_out = dram.tile([N_CORES, 256, 512], FP)

            ones = consts.tile([128, 1], FP)
            nc.gpsimd.memset(ones[:], 1.0)
            # additive causal masks for the 4 diagonal offsets:
            # masks[p, i, q] = 0 if q >= p + i*128 else NEG
            masks = consts.tile([128, 4, 512], FP)
            nc.gpsimd.memset(masks[:], 0.0)
            for i in range(4):
                nc.gpsimd.affine_select(
                    out=masks[:, i, :],
                    in_=masks[:, i, :],
                    compare_op=mybir.AluOpType.is_ge,
                    fill=NEG,
                    base=-(i * 128),
                    pattern=[[1, 512]],
                    channel_multiplier=-1,
                )

            kT = persist.tile([128, 2, T], FP)       # [dh, hl, t]
            v_sb = persist.tile([128, 32, 256], FP)  # [t%128, t//128, head_feat]

            # ---------------- Phase 1: Q/K/V projections ----------------
            with (
                tc.tile_pool(name="wpool", bufs=1) as wpool,
                tc.tile_pool(name="xtp", bufs=6) as xtp,
                tc.tile_pool(name="qstage", bufs=3) as qstage,
                tc.tile_pool(name="ps1", bufs=1, space="PSUM") as ps1,
            ):
                wq_sb = wpool.tile([128, KK, 256], FP)
                wk_sb = wpool.tile([128, KK, 256], FP)
                wv_sb = wpool.tile([128, KK, 256], FP)
                nc.sync.dma_start(wq_sb[:], wq[:])
                nc.sync.dma_start(wk_sb[:], wk[:])
                nc.sync.dma_start(wv_sb[:], wv[:])

                for strip in range(NSTRIP):
                    t0 = strip * 512
                    xq = []
                    for qtr in range(4):
                        xtile = xtp.tile([128, 4, 512], FP, tag="xt")
                        nc.sync.dma_start(
                            xtile[:], xt[:, qtr * 4 : (qtr + 1) * 4, t0 : t0 + 512]
                        )
                        xq.append(xtile)

                    qk_ps = [ps1.tile([128, 512], FP, tag=f"qk{j}", name=f"qk_ps{j}") for j in range(4)]
                    v_ps = [ps1.tile([128, 256], FP, tag=f"vp{j}", name=f"v_ps{j}") for j in range(4)]
                    for kk in range(KK):
                        xsl = xq[kk // 4][:, kk % 4, :]
                        st, sp = kk == 0, kk == KK - 1
                        nc.tensor.matmul(qk_ps[0][:], wq_sb[:, kk, 0:128], xsl, start=st, stop=sp)
                        nc.tensor.matmul(qk_ps[1][:], wq_sb[:, kk, 128:256], xsl, start=st, stop=sp)
                        nc.tensor.matmul(qk_ps[2][:], wk_sb[:, kk, 0:128], xsl, start=st, stop=sp)
                        nc.tensor.matmul(qk_ps[3][:], wk_sb[:, kk, 128:256], xsl, start=st, stop=sp)
                        for tt in range(4):
                            nc.tensor.matmul(
                                v_ps[tt][:],
                                xsl[:, tt * 128 : (tt + 1) * 128],
                                wv_sb[:, kk, :],
                                start=st,
                                stop=sp,
                            )
                    for hl in range(2):
                        qs = qstage.tile([128, 512], FP, tag="qs")
                        nc.scalar.copy(qs[:], qk_ps[hl][:])
                        nc.sync.dma_start(qt_d[hl, :, t0 : t0 + 512], qs[:])
                        nc.scalar.copy(kT[:, hl, t0 : t0 + 512], qk_ps[2 + hl][:])
                    for tt in range(4):
                        nc.vector.tensor_copy(v_sb[:, strip * 4 + tt, :], v_ps[tt][:])

            # ---------------- Phase 2: causal attention ----------------
            with tc.tile_pool(name="wop", bufs=8) as wop:
                # prefetch output-projection weights during attention
                wo_tiles = []
                for dd in range(KK):
                    wod = wop.tile([128, KK, 128], FP, tag="wod")
                    nc.sync.dma_start(wod[:], wo[:, :, dd * 128 : (dd + 1) * 128])
                    wo_tiles.append(wod)

                with (
                    tc.tile_pool(name="qtp", bufs=3) as qtp,
                    tc.tile_pool(name="expp", bufs=3) as expp,
                    tc.tile_pool(name="smallp", bufs=2) as smallp,
                    tc.tile_pool(name="otp", bufs=3) as otp,
                    tc.tile_pool(name="psT", bufs=3, space="PSUM") as psT,
                    tc.tile_pool(name="psA", bufs=2, space="PSUM") as psA,
                    tc.tile_pool(name="psS", bufs=2, space="PSUM") as psS,
                ):
                    self_attention(tc, nc, qt_d, a2a_in, kT, v_sb, ones, masks,
                                   qtp, expp, smallp, otp, psT, psA, psS)

                if dbg:
                    nc.sync.dma_start(dbg["q"][:], qt_d[:])
                    nc.sync.dma_start(dbg["k"][:], kT[:])
                    nc.sync.dma_start(dbg["v"][:], v_sb[:])
                    nc.sync.dma_start(dbg["ain"][:], a2a_in[:])
                    nc.sync.dma_start(dbg["masks"][:], masks[:])

                # ---------------- Phase 3: AllToAll ----------------
                nc.gpsimd.collective_compute(
                    "AllToAll",
                    mybir.AluOpType.bypass,
                    replica_groups=[list(range(N_CORES))],
                    ins=[a2a_in[:].opt()],
                    outs=[a2a_out[:].opt()],
                )

                if dbg:
                    nc.sync.dma_start(dbg["aout"][:], a2a_out[:])

                # ---------------- Phase 4: output projection ----------------
                with (
                    tc.tile_pool(name="otsb", bufs=1) as otsb_pool,
                    tc.tile_pool(name="outp", bufs=3) as outp,
                    tc.tile_pool(name="ps4", bufs=2, space="PSUM") as ps4,
                ):
                    ot_sb = otsb_pool.tile([128, KK, 512], FP)
                    nc.sync.dma_start(
                        ot_sb[:],
                        a2a_out[:].rearrange("i f t -> (i f) t").rearrange(
                            "(ff p) t -> p ff t", p=128
                        ),
                    )
                    for dd in range(KK):
                        op = ps4.tile([128, 512], FP, tag="op")
                        for ff in range(KK):
                            nc.tensor.matmul(
                                op[:],
                                wo_tiles[dd][:, ff, :],
                                ot_sb[:, ff, :],
                                start=(ff == 0),
                                stop=(ff == KK - 1),
                            )
                        ob = outp.tile([128, 512], FP, tag="ob")
                        nc.scalar.copy(ob[:], op[:])
                        nc.sync.dma_start(out_t[dd * 128 : (dd + 1) * 128, :], ob[:])

    nc.compile()
    return nc


def self_attention(tc, nc, qt_d, a2a_in, kT, v_sb, ones, masks,
                   qtp, expp, smallp, otp, psT, psA, psS):
    for b in range(B):
        for hl in range(2):
            for s in range(4):
                q0 = b * S + s * 512
                qts = qtp.tile([128, 512], FP, tag="qts")
                nc.sync.dma_start(qts[:], qt_d[hl, :, q0 : q0 + 512])
                avp = psA.tile([128, 512], FP, tag="av")
                smp = psS.tile([1, 512], FP, tag="sm")
                nk = 4 * (s + 1)
                for ki in range(nk):
                    stp = psT.tile([128, 512], FP, tag="st")
                    nc.tensor.matmul(
                        stp[:],
                        kT[:, hl, b * S + ki * 128 : b * S + (ki + 1) * 128],
                        qts[:],
                        start=True,
                        stop=True,
                    )
                    if ki >= 4 * s:
                        nc.vector.tensor_add(stp[:], stp[:], masks[:, ki - 4 * s, :])
                    ex = expp.tile([128, 512], FP, tag="ex")
                    nc.scalar.activation(ex[:], stp[:], mybir.ActivationFunctionType.Exp)
                    st, sp = ki == 0, ki == nk - 1
                    nc.tensor.matmul(
                        avp[:],
                        v_sb[:, b * 16 + ki, hl * 128 : (hl + 1) * 128],
                        ex[:],
                        start=st,
                        stop=sp,
                    )
                    nc.tensor.matmul(smp[:], ones[:], ex[:], start=st, stop=sp)
                sums_sb = smallp.tile([1, 512], FP, tag="sums")
                nc.scalar.copy(sums_sb[:], smp[:])
                sbc = smallp.tile([128, 512], FP, tag="sbc")
                nc.gpsimd.partition_broadcast(sbc[:], sums_sb[:])
                rbc = smallp.tile([128, 512], FP, tag="rbc")
                nc.vector.reciprocal(rbc[:], sbc[:])
                ot = otp.tile([128, 512], FP, tag="ot")
                nc.vector.tensor_mul(ot[:], avp[:], rbc[:])
                j = b * 4 + s
                nc.sync.dma_start(a2a_in[j, hl * 128 : (hl + 1) * 128, :], ot[:])


_NC_CACHE = None


def _get_nc():
    global _NC_CACHE
    if _NC_CACHE is None:
        _NC_CACHE = build_nc()
    return _NC_CACHE


def _make_in_maps(x, wq, wk, wv, wo):
    x = np.ascontiguousarray(np.asarray(x, dtype=np.float32))
    wq = np.asarray(wq, dtype=np.float32)
    wk = np.asarray(wk, dtype=np.float32)
    wv = np.asarray(wv, dtype=np.float32)
    wo = np.asarray(wo, dtype=np.float32)

    x_flat = x.reshape(T, D)
    # xt[p, kk, t] = x_flat[t, kk*128+p]
    xt_host = np.ascontiguousarray(x_flat.T.reshape(KK, 128, T).transpose(1, 0, 2))
    # wo_dev[p, ff, d] = wo[d, ff*128+p]
    wo_host = np.ascontiguousarray(wo.T.reshape(KK, 128, D).transpose(1, 0, 2))
    scale = 1.0 / np.sqrt(np.float32(DH))

    in_maps = []
    for c in range(N_CORES):
        sl = slice(c * 256, (c + 1) * 256)

        def wslice(w, scaled=False):
            wc = w[sl, :].T  # [D, 256]
            if scaled:
                wc = wc * scale
            return np.ascontiguousarray(wc.reshape(KK, 128, 256).transpose(1, 0, 2))

        in_maps.append(
            {
                "xt": xt_host,
                "wq": wslice(wq, scaled=True),
                "wk": wslice(wk),
                "wv": wslice(wv),
                "wo": wo_host,
            }
        )
    return in_maps


def _run(x, wq, wk, wv, wo, trace=False):
    nc = _get_nc()
    in_maps = _make_in_maps(x, wq, wk, wv, wo)
    res = run_bass_kernel_spmd(nc, in_maps, list(range(N_CORES)), trace=trace)
    rows = [res.results[c]["out_t"].T for c in range(N_CORES)]  # [512, D] each
    out = np.concatenate(rows, axis=0).reshape(B, S, D)
    return out, res


def kernel(x, wq, wk, wv, wo):
    out, _ = _run(x, wq, wk, wv, wo, trace=False)
    return out


# revision 11
# speedup vs baseline: 3.3660x; 3.3660x over previous
"""Context-parallel causal attention block on 8 Trainium2 NeuronCores.

Strategy: tensor-parallel split-heads. Each core c computes Q/K/V projections
for its 2 heads (of 16) over all tokens with host-sliced weights, runs causal
attention locally (feature-major layouts, no transposes), then one on-device
AllToAll re-shards from head-parallel to token-parallel, and each core runs the
output projection for its 512-token row slice. Host concatenates row slices.

Matmul operands are bf16 (fp32 matmuls are two-pass / half-rate on TRN2's PE);
all accumulation stays fp32 in PSUM, softmax runs on fp32 scores.
"""
import sys

sys.path.insert(0, "/opt/trn_rl_repo")

import ml_dtypes
import numpy as np

import concourse.bass as bass
import concourse.tile as tile
from concourse import bacc, mybir
from concourse.bass_utils import run_bass_kernel_spmd

FP = mybir.dt.float32
BF = mybir.dt.bfloat16
NPBF = ml_dtypes.bfloat16
N_CORES = 8
B, S, D, H, DH = 2, 2048, 2048, 16, 128
T = B * S            # 4096 flattened tokens, b-major
KK = D // 128        # 16 contraction k-tiles
NSTRIP = T // 512    # 8 token strips of 512
ROWS = T // N_CORES  # 512 output rows per core
NEG = -1.0e30


def build_nc(debug_taps: bool = False) -> bacc.Bacc:
    nc = bacc.Bacc("TRN2", target_bir_lowering=False, debug=False, num_devices=N_CORES)

    xt = nc.dram_tensor("xt", [128, KK, T], BF, kind="ExternalInput")
    wq = nc.dram_tensor("wq", [128, KK, 256], BF, kind="ExternalInput")
    wk = nc.dram_tensor("wk", [128, KK, 256], BF, kind="ExternalInput")
    wv = nc.dram_tensor("wv", [128, KK, 256], BF, kind="ExternalInput")
    wo = nc.dram_tensor("wo", [128, KK, D], BF, kind="ExternalInput")
    out_t = nc.dram_tensor("out_t", [D, ROWS], FP, kind="ExternalOutput")
    dbg = {}
    if debug_taps:
        dbg["q"] = nc.dram_tensor("dbg_q", [128, 2, T], BF, kind="ExternalOutput")
        dbg["k"] = nc.dram_tensor("dbg_k", [128, 2, T], BF, kind="ExternalOutput")
        dbg["v"] = nc.dram_tensor("dbg_v", [128, 32, 256], BF, kind="ExternalOutput")
        dbg["ain"] = nc.dram_tensor("dbg_ain", [N_CORES, 256, 512], BF, kind="ExternalOutput")
        dbg["aout"] = nc.dram_tensor("dbg_aout", [N_CORES, 256, 512], BF, kind="ExternalOutput")

    with tile.TileContext(nc) as tc:
        with (
            tc.tile_pool(name="dram", bufs=1, space="DRAM") as dram,
            tc.tile_pool(name="consts", bufs=1) as consts,
            tc.tile_pool(name="persist", bufs=1) as persist,
        ):
            a2a_in = dram.tile([N_CORES, 256, 512], BF)  # [dest, feat, tok]
            a2a_out = dram.tile([N_CORES, 256, 512], BF)

            ones = consts.tile([128, 1], BF)
            nc.gpsimd.memset(ones[:], 1.0)
            # additive causal masks for the 4 diagonal offsets:
            # masks[p, i, q] = 0 if q >= p + i*128 else NEG
            masks = consts.tile([128, 4, 512], FP)
            nc.gpsimd.memset(masks[:], 0.0)
            for i in range(4):
                nc.gpsimd.affine_select(
                    out=masks[:, i, :],
                    in_=masks[:, i, :],
                    compare_op=mybir.AluOpType.is_ge,
                    fill=NEG,
                    base=-(i * 128),
                    pattern=[[1, 512]],
                    channel_multiplier=-1,
                )

            qT = persist.tile([128, 2, T], BF)       # [dh, hl, t]
            kT = persist.tile([128, 2, T], BF)
            v_sb = persist.tile([128, 32, 256], BF)  # [t%128, t//128, head_feat]

            # ---------------- Phase 1: Q/K/V projections ----------------
            with (
                tc.tile_pool(name="wpool", bufs=1) as wpool,
                tc.tile_pool(name="xtp", bufs=6) as xtp,
                tc.tile_pool(name="ps1", bufs=1, space="PSUM") as ps1,
            ):
                wq_sb = wpool.tile([128, KK, 256], BF)
                wk_sb = wpool.tile([128, KK, 256], BF)
                wv_sb = wpool.tile([128, KK, 256], BF)
                nc.sync.dma_start(wq_sb[:], wq[:])
                nc.sync.dma_start(wk_sb[:], wk[:])
                nc.sync.dma_start(wv_sb[:], wv[:])

                for strip in range(NSTRIP):
                    t0 = strip * 512
                    xq = []
                    for qtr in range(4):
                        xtile = xtp.tile([128, 4, 512], BF, tag="xt")
                        nc.sync.dma_start(
                            xtile[:], xt[:, qtr * 4 : (qtr + 1) * 4, t0 : t0 + 512]
                        )
                        xq.append(xtile)

                    qk_ps = [ps1.tile([128, 512], FP, tag=f"qk{j}", name=f"qk_ps{j}") for j in range(4)]
                    v_ps = [ps1.tile([128, 256], FP, tag=f"vp{j}", name=f"v_ps{j}") for j in range(4)]
                    for kk in range(KK):
                        xsl = xq[kk // 4][:, kk % 4, :]
                        st, sp = kk == 0, kk == KK - 1
                        nc.tensor.matmul(qk_ps[0][:], wq_sb[:, kk, 0:128], xsl, start=st, stop=sp)
                        nc.tensor.matmul(qk_ps[1][:], wq_sb[:, kk, 128:256], xsl, start=st, stop=sp)
                        nc.tensor.matmul(qk_ps[2][:], wk_sb[:, kk, 0:128], xsl, start=st, stop=sp)
                        nc.tensor.matmul(qk_ps[3][:], wk_sb[:, kk, 128:256], xsl, start=st, stop=sp)
                        for tt in range(4):
                            nc.tensor.matmul(
                                v_ps[tt][:],
                                xsl[:, tt * 128 : (tt + 1) * 128],
                                wv_sb[:, kk, :],
                                start=st,
                                stop=sp,
                            )
                    for hl in range(2):
                        nc.scalar.copy(qT[:, hl, t0 : t0 + 512], qk_ps[hl][:])
                        nc.scalar.copy(kT[:, hl, t0 : t0 + 512], qk_ps[2 + hl][:])
                    for tt in range(4):
                        nc.vector.tensor_copy(v_sb[:, strip * 4 + tt, :], v_ps[tt][:])

            # ---------------- Phase 2: causal attention ----------------
            with tc.tile_pool(name="wop", bufs=8) as wop:
                # prefetch output-projection weights during attention
                wo_tiles = []
                for dd in range(KK):
                    wod = wop.tile([128, KK, 128], BF, tag="wod")
                    nc.sync.dma_start(wod[:], wo[:, :, dd * 128 : (dd + 1) * 128])
                    wo_tiles.append(wod)

                with (
                    tc.tile_pool(name="expp", bufs=3) as expp,
                    tc.tile_pool(name="smallp", bufs=2) as smallp,
                    tc.tile_pool(name="otp", bufs=3) as otp,
                    tc.tile_pool(name="psT", bufs=3, space="PSUM") as psT,
                    tc.tile_pool(name="psA", bufs=2, space="PSUM") as psA,
                    tc.tile_pool(name="psS", bufs=2, space="PSUM") as psS,
                ):
                    self_attention(tc, nc, a2a_in, qT, kT, v_sb, ones, masks,
                                   expp, smallp, otp, psT, psA, psS)

                if dbg:
                    nc.sync.dma_start(dbg["q"][:], qT[:])
                    nc.sync.dma_start(dbg["k"][:], kT[:])
                    nc.sync.dma_start(dbg["v"][:], v_sb[:])
                    nc.sync.dma_start(dbg["ain"][:], a2a_in[:])

                # ---------------- Phase 3: AllToAll ----------------
                nc.gpsimd.collective_compute(
                    "AllToAll",
                    mybir.AluOpType.bypass,
                    replica_groups=[list(range(N_CORES))],
                    ins=[a2a_in[:].opt()],
                    outs=[a2a_out[:].opt()],
                )
                if dbg:
                    nc.sync.dma_start(dbg["aout"][:], a2a_out[:])

                # ---------------- Phase 4: output projection ----------------
                with (
                    tc.tile_pool(name="otsb", bufs=1) as otsb_pool,
                    tc.tile_pool(name="outp", bufs=3) as outp,
                    tc.tile_pool(name="ps4", bufs=2, space="PSUM") as ps4,
                ):
                    ot_sb = otsb_pool.tile([128, KK, 512], BF)
                    nc.sync.dma_start(
                        ot_sb[:],
                        a2a_out[:].rearrange("i f t -> (i f) t").rearrange(
                            "(ff p) t -> p ff t", p=128
                        ),
                    )
                    for dd in range(KK):
                        op = ps4.tile([128, 512], FP, tag="op")
                        for ff in range(KK):
                            nc.tensor.matmul(
                                op[:],
                                wo_tiles[dd][:, ff, :],
                                ot_sb[:, ff, :],
                                start=(ff == 0),
                                stop=(ff == KK - 1),
                            )
                        ob = outp.tile([128, 512], FP, tag="ob")
                        nc.scalar.copy(ob[:], op[:])
                        nc.sync.dma_start(out_t[dd * 128 : (dd + 1) * 128, :], ob[:])

    nc.compile()
    return nc


def self_attention(tc, nc, a2a_in, qT, kT, v_sb, ones, masks,
                   expp, smallp, otp, psT, psA, psS):
    for b in range(B):
        for hl in range(2):
            for s in range(4):
                q0 = b * S + s * 512
                qts = qT[:, hl, q0 : q0 + 512]
                avp = psA.tile([128, 512], FP, tag="av")
                smp = psS.tile([1, 512], FP, tag="sm")
                nk = 4 * (s + 1)
                for ki in range(nk):
                    stp = psT.tile([128, 512], FP, tag="st")
                    nc.tensor.matmul(
                        stp[:],
                        kT[:, hl, b * S + ki * 128 : b * S + (ki + 1) * 128],
                        qts,
                        start=True,
                        stop=True,
                    )
                    if ki >= 4 * s:
                        nc.vector.tensor_add(stp[:], stp[:], masks[:, ki - 4 * s, :])
                    ex = expp.tile([128, 512], BF, tag="ex")
                    nc.scalar.activation(ex[:], stp[:], mybir.ActivationFunctionType.Exp)
                    st, sp = ki == 0, ki == nk - 1
                    nc.tensor.matmul(
                        avp[:],
                        v_sb[:, b * 16 + ki, hl * 128 : (hl + 1) * 128],
                        ex[:],
                        start=st,
                        stop=sp,
                    )
                    nc.tensor.matmul(smp[:], ones[:], ex[:], start=st, stop=sp)
                sums_sb = smallp.tile([1, 512], FP, tag="sums")
                nc.scalar.copy(sums_sb[:], smp[:])
                sbc = smallp.tile([128, 512], FP, tag="sbc")
                nc.gpsimd.partition_broadcast(sbc[:], sums_sb[:])
                rbc = smallp.tile([128, 512], FP, tag="rbc")
                nc.vector.reciprocal(rbc[:], sbc[:])
                ot = otp.tile([128, 512], BF, tag="ot")
                nc.vector.tensor_mul(ot[:], avp[:], rbc[:])
                j = b * 4 + s
                nc.sync.dma_start(a2a_in[j, hl * 128 : (hl + 1) * 128, :], ot[:])


_NC_CACHE = {}


def _get_nc(debug_taps=False):
    key = bool(debug_taps)
    if key not in _NC_CACHE:
        _NC_CACHE[key] = build_nc(debug_taps=key)
    return _NC_CACHE[key]


def _make_in_maps(x, wq, wk, wv, wo):
    x = np.ascontiguousarray(np.asarray(x, dtype=np.float32))
    wq = np.asarray(wq, dtype=np.float32)
    wk = np.asarray(wk, dtype=np.float32)
    wv = np.asarray(wv, dtype=np.float32)
    wo = np.asarray(wo, dtype=np.float32)

    x_flat = x.reshape(T, D)
    # xt[p, kk, t] = x_flat[t, kk*128+p]
    xt_host = np.ascontiguousarray(
        x_flat.T.reshape(KK, 128, T).transpose(1, 0, 2)
    ).astype(NPBF)
    # wo_dev[p, ff, d] = wo[d, ff*128+p]
    wo_host = np.ascontiguousarray(
        wo.T.reshape(KK, 128, D).transpose(1, 0, 2)
    ).astype(NPBF)
    scale = 1.0 / np.sqrt(np.float32(DH))

    in_maps = []
    for c in range(N_CORES):
        sl = slice(c * 256, (c + 1) * 256)

        def wslice(w, scaled=False):
            wc = w[sl, :].T  # [D, 256]
            if scaled:
                wc = wc * scale
            return np.ascontiguousarray(
                wc.reshape(KK, 128, 256).transpose(1, 0, 2)
            ).astype(NPBF)

        in_maps.append(
            {
                "xt": xt_host,
                "wq": wslice(wq, scaled=True),
                "wk": wslice(wk),
                "wv": wslice(wv),
                "wo": wo_host,
            }
        )
    return in_maps


def _run(x, wq, wk, wv, wo, trace=False):
    nc = _get_nc()
    in_maps = _make_in_maps(x, wq, wk, wv, wo)
    res = run_bass_kernel_spmd(nc, in_maps, list(range(N_CORES)), trace=trace)
    rows = [res.results[c]["out_t"].T for c in range(N_CORES)]  # [512, D] each
    out = np.concatenate(rows, axis=0).reshape(B, S, D)
    return out, res


def kernel(x, wq, wk, wv, wo):
    out, _ = _run(x, wq, wk, wv, wo, trace=False)
    return out
